# revision 21
# baseline (speedup 1.0000x reference)
"""Bass/Trainium2 kernel for nn_BilinearInteractionLayer.

Computes, for all field pairs (i, j) with i < j (P = C(32,2) = 496 pairs):
    out[b, p, :] = (emb[b, i_p, :] @ W[p].T) * emb[b, j_p, :]
with emb [2048, 32, 64] fp32 and W [496, 64, 64] fp32.

Strategy: data-parallel over batch across 8 cores (B=256 per core, two
128-row chunks), W replicated. Per (chunk, field) the pairs (f, j), j>f are
contiguous in the global pair order, so matmuls compute
proj[b, (j, e)] = X_f[b, :] @ Wcat_f.T with the batch chunk (128 rows) as
the PE stationary operand and the stacked pair weights streaming (K=64).

Fields are assigned to PE row groups by parity (even fields on partitions
0:64 / row group h0, odd on 64:128 / h1) and adjacent-field matmuls are
zipped per-instruction, so each matmul's drain overlaps the next one's
stream on the other row group (~0.71 ns/col vs 0.83 serialized). The
parity split also means X and W each live on exactly one partition half --
no duplicated operands.

The per-chunk output stream (31744 fp16 cols) is tiled into uniform
1024-col PSUM windows (2 banks, 4 in flight) and 2048-col SBUF stage tiles
(= 32 pairs, one output DMA each on the SP ring). Window evictions fuse the
v_j multiply and are load-balanced over three paths: ACT copy (PSUM->fp16)
+ DVE 16-bit multiply (2x mode), DVE direct from PSUM (1x), and ACT copy +
Pool 16-bit multiply (GPSIMD cannot read PSUM). Input loads are chunked
field-aligned and priority-ordered across the SP/ACT/Pool DGE rings so the
first matmul's operands land ~2us after the rings open; output stores then
keep the 16 DMA engines saturated.

Matmul operands are cast to fp16 on the host (rel err ~3e-4 with fp32 PSUM
accumulation); output is written to HBM as fp16 and upcast to fp32 on the
host during the gather.
"""

import sys

sys.path.insert(0, "/opt/trn_rl_repo")

from contextlib import ExitStack

import numpy as np

import concourse.bass as bass
import concourse.tile as tile
from concourse import bacc, bass_utils, mybir
from concourse._compat import with_exitstack

NUM_FIELDS = 32
EMB_DIM = 64
BATCH = 2048
N_CORES = 8
B_CORE = BATCH // N_CORES          # 256
N_BCHUNK = B_CORE // 128           # 2
P_TOTAL = NUM_FIELDS * (NUM_FIELDS - 1) // 2  # 496
TC = P_TOTAL * EMB_DIM             # 31744 out cols per b-chunk

# OFF[f] = global pair index of first pair (f, f+1); OFF[32] = 496
OFF = [0] * (NUM_FIELDS + 1)
for _f in range(1, NUM_FIELDS + 1):
    OFF[_f] = OFF[_f - 1] + (NUM_FIELDS - _f)


def _cols(f):
    return (NUM_FIELDS - 1 - f) * EMB_DIM


WIN = 1024             # psum window cols (2 PSUM banks fp32)
STAGE = 4096           # stage cols = 64 pairs, 8KB store descriptors
N_WIN = (TC + WIN - 1) // WIN        # 31 per chunk
N_STAGE = (TC + STAGE - 1) // STAGE  # 16 per chunk (last = 1024)
MM_N = 512             # matmul piece <= one PSUM bank

# Field-aligned wt DMA groups; each group is one SBUF tile [128, gcols]
# with the group's even fields packed on partitions 0:64 and odd fields on
# 64:128 (per-plane concatenation, padded to the wider plane). Group 0
# {0, 1} lives inside the "prefix" tile behind the chunk-0 embt head block
# (one tensor -> large descriptors -> fast first-matmul gating; per-packet
# DMA round-robin gives each queue bandwidth proportional to its
# descriptor size, so small prefix descriptors would starve).
WT_GROUPS = [
    [0, 1],
    [2, 3],
    [4, 5, 6, 7],
    [8, 9, 10, 11, 12, 13],
    [14, 15, 16, 17, 18, 19, 20, 21],
    list(range(22, 32)),
]
# Per-field tile index / plane / local col; per-group tile width.
WT_TILE = {}
WT_LCOL = {}
WT_GCOLS = []
WT_BASE = []            # col offset of W data within the group tile
for _k, _g in enumerate(WT_GROUPS):
    base = 128 if _k == 0 else 0
    lc = [0, 0]
    for _f in _g:
        _p = _f % 2
        WT_TILE[_f] = _k
        WT_LCOL[_f] = lc[_p]
        lc[_p] += _cols(_f)
    WT_BASE.append(base)
    WT_GCOLS.append(base + max(lc))

EMBT_HEAD_BLOCKS = 1   # chunk-0 embt head (fields 0, 1) inside the prefix
PREFIX_A_COLS = EMBT_HEAD_BLOCKS * 128 + MM_N  # gates the first zip pieces

# Matmul zip pairs: (even, odd) adjacent pairs, last field alone.
PAIRS_ZIP = [(f, f + 1) for f in range(0, NUM_FIELDS - 2, 2)] + [
    (NUM_FIELDS - 2,)
]

# Eviction path per window, cycled: A = ACT copy + DVE 16-bit mul,
# D = DVE direct from PSUM, P = ACT copy + Pool 16-bit mul. Measured
# per-col engine costs incl. per-instruction overheads (ns): ACT copy
# .91, DVE 2x mul .83, DVE direct 1.9, Pool mul 2.4 -> balance at
# ~57% A / 14% D / 29% P (each engine ~50us).
PATH_PATTERN = "AAPADAP"

OUT_DT = mybir.dt.float16


def _field_of(col):
    for f in range(NUM_FIELDS - 1):
        if col < OFF[f + 1] * EMB_DIM:
            return f
    raise AssertionError(col)


def _field_pieces(f, grid):
    """Split field f's output-col range at field-relative `grid` multiples
    (PSUM tiles are field-aligned, so banks are too). Yields (a, b)."""
    c0, c1 = OFF[f] * EMB_DIM, OFF[f + 1] * EMB_DIM
    cs = list(range(c0, c1, grid)) + [c1]
    return list(zip(cs, cs[1:]))


def _stage_split(a, b):
    """Split [a, b) at absolute stage boundaries. Yields (x, y)."""
    cuts = {a, b}
    g = (a // STAGE + 1) * STAGE
    while g < b:
        cuts.add(g)
        g += STAGE
    cs = sorted(cuts)
    return list(zip(cs, cs[1:]))


@with_exitstack
def _bilinear_kernel(
    ctx: ExitStack,
    tc: "tile.TileContext",
    out_ap: bass.AP,
    wt_aps,
    embt_aps,   # embt0h, embt0t, embt1
    embn_aps,   # embn0, embn1
):
    nc = tc.nc

    const_pool = ctx.enter_context(tc.tile_pool(name="const", bufs=1))
    psum_pool = ctx.enter_context(tc.tile_pool(name="psum", bufs=4, space="PSUM"))
    stage_pool = ctx.enter_context(tc.tile_pool(name="stage", bufs=8))
    tmp_pool = ctx.enter_context(tc.tile_pool(name="tmp", bufs=6))

    embt0t = const_pool.tile([128, (NUM_FIELDS // 2 - EMBT_HEAD_BLOCKS) * 128],
                             mybir.dt.float16, tag="e0t", name="embt0t")
    embt1 = const_pool.tile([128, NUM_FIELDS // 2 * 128], mybir.dt.float16,
                            tag="e1", name="embt1")
    embn_tiles = [
        const_pool.tile(
            [128, NUM_FIELDS * EMB_DIM], mybir.dt.float16, tag=f"en{c}",
            name=f"embn{c}",
        )
        for c in range(N_BCHUNK)
    ]
    wt_tiles = [
        const_pool.tile([128, WT_GCOLS[k]], mybir.dt.float16, tag=f"wt{k}",
                        name=f"wtt{k}")
        for k in range(len(WT_GROUPS))
    ]

    # Input loads, priority-ordered by first-use deadline. The prefix tile
    # (chunk-0 embt head + W fields {0,1}) goes first on the otherwise-empty
    # SP ring, split so a small leading DMA gates the first zip pieces; the
    # remaining groups are ordered so each lands just before the PE's
    # field-order consumption reaches it. SP then carries only output
    # stores.
    nc.sync.dma_start(wt_tiles[0][:, 0:PREFIX_A_COLS],
                      wt_aps[0][:, 0:PREFIX_A_COLS])
    nc.sync.dma_start(embn_tiles[0][0:64, :], embn_aps[0][0:64, :])
    nc.scalar.dma_start(wt_tiles[0][:, PREFIX_A_COLS:],
                      wt_aps[0][:, PREFIX_A_COLS:])
    nc.scalar.dma_start(embn_tiles[0][64:128, :], embn_aps[0][64:128, :])
    nc.gpsimd.dma_start(wt_tiles[1][:], wt_aps[1][:])
    nc.gpsimd.dma_start(embt0t[:], embt_aps[0][:])
    nc.gpsimd.dma_start(wt_tiles[2][:], wt_aps[2][:])
    nc.scalar.dma_start(wt_tiles[3][:], wt_aps[3][:])
    nc.gpsimd.dma_start(wt_tiles[4][:], wt_aps[4][:])
    nc.gpsimd.dma_start(wt_tiles[5][:], wt_aps[5][:])
    nc.gpsimd.dma_start(embt1[:], embt_aps[1][:])
    nc.scalar.dma_start(embn_tiles[1][:], embn_aps[1][:])

    def lhsT(c, f):
        r0 = 64 * (f % 2)
        blk = f // 2
        if c == 1:
            t = embt1
        elif blk < EMBT_HEAD_BLOCKS:
            t = wt_tiles[0]
        else:
            t = embt0t
            blk -= EMBT_HEAD_BLOCKS
        return t[r0 : r0 + 64, blk * 128 : (blk + 1) * 128]

    def rhs(f, a, b):
        k = WT_TILE[f]
        r0 = 64 * (f % 2)
        lc = WT_BASE[k] + WT_LCOL[f] + (a - OFF[f] * EMB_DIM)
        return wt_tiles[k][r0 : r0 + 64, lc : lc + (b - a)]

    widx = [0]

    for c in range(N_BCHUNK):
        # PSUM tiles are per (field, 1024-col field-window) so each tile —
        # and so each PSUM bank — is written by exactly one PE row group.
        fw_ps = {}
        fw_fill = {}
        stg_fill = [0] * N_STAGE
        stg_tile = {}

        def evict_fw(f, a0, a1, ps, c=c, stg_fill=stg_fill, stg_tile=stg_tile):
            wlen = a1 - a0
            path = PATH_PATTERN[widx[0] % len(PATH_PATTERN)]
            widx[0] += 1
            if path in ("A", "P"):
                tmp = tmp_pool.tile([128, WIN], mybir.dt.float16, tag="ev",
                                    name="ev")
                nc.scalar.copy(tmp[:, 0:wlen], ps[:, 0:wlen])
            for a, b in _stage_split(a0, a1):
                e0 = (f + 1) * EMB_DIM + (a - OFF[f] * EMB_DIM)
                in1 = embn_tiles[c][:, e0 : e0 + (b - a)]
                s = a // STAGE
                if s not in stg_tile:
                    stg_tile[s] = stage_pool.tile([128, STAGE], OUT_DT,
                                                  tag="stage", name="stg")
                dst = stg_tile[s][:, a - s * STAGE : b - s * STAGE]
                if path == "A":
                    nc.vector.tensor_mul(dst, tmp[:, a - a0 : b - a0], in1)
                elif path == "D":
                    nc.vector.tensor_mul(dst, ps[:, a - a0 : b - a0], in1)
                else:
                    nc.gpsimd.tensor_mul(dst, tmp[:, a - a0 : b - a0], in1)
                stg_fill[s] += b - a
                s0 = s * STAGE
                scols = min(STAGE, TC - s0)
                if stg_fill[s] == scols:
                    npair = scols // EMB_DIM
                    p0 = s * (STAGE // EMB_DIM)
                    nc.sync.dma_start(
                        out_ap[c * 128 : (c + 1) * 128, p0 : p0 + npair, :],
                        stg_tile.pop(s)[:, 0:scols],
                    )

        def emit_piece(f, a, b, c=c, fw_ps=fw_ps, fw_fill=fw_fill):
            fc0 = OFF[f] * EMB_DIM
            k = (a - fc0) // WIN
            a0 = fc0 + k * WIN
            a1 = min(a0 + WIN, OFF[f + 1] * EMB_DIM)
            assert b <= a1
            if (f, k) not in fw_ps:
                fw_ps[(f, k)] = psum_pool.tile([128, WIN], mybir.dt.float32,
                                               tag="ps", name="ps")
                fw_fill[(f, k)] = 0
            nc.tensor.matmul(
                fw_ps[(f, k)][:, a - a0 : b - a0],
                lhsT(c, f),
                rhs(f, a, b),
                start=True,
                stop=True,
            )
            fw_fill[(f, k)] += b - a
            if fw_fill[(f, k)] == a1 - a0:
                evict_fw(f, a0, a1, fw_ps.pop((f, k)))

        for zp in PAIRS_ZIP:
            lists = [_field_pieces(f, MM_N) for f in zp]
            n = max(len(x) for x in lists)
            for i in range(n):
                for f, pl in zip(zp, lists):
                    if i < len(pl):
                        emit_piece(f, *pl[i])

        assert not fw_ps and not stg_tile, (fw_ps.keys(), stg_tile.keys())


_CACHE = {}


def _get_program():
    if "nc" not in _CACHE:
        nc = bacc.Bacc(
            "TRN2", target_bir_lowering=False, debug=False, num_devices=N_CORES
        )
        wt_aps = [
            nc.dram_tensor(
                f"wt{k}", [128, WT_GCOLS[k]], mybir.dt.float16,
                kind="ExternalInput",
            ).ap()
            for k in range(len(WT_GROUPS))
        ]
        embt_aps = [
            nc.dram_tensor(
                "embt0t",
                [128, (NUM_FIELDS // 2 - EMBT_HEAD_BLOCKS) * 128],
                mybir.dt.float16, kind="ExternalInput",
            ).ap(),
            nc.dram_tensor("embt1", [128, NUM_FIELDS // 2 * 128],
                           mybir.dt.float16, kind="ExternalInput").ap(),
        ]
        embn_aps = [
            nc.dram_tensor(
                f"embn{c}", [128, NUM_FIELDS * EMB_DIM], mybir.dt.float16,
                kind="ExternalInput",
            ).ap()
            for c in range(N_BCHUNK)
        ]
        out_ap = nc.dram_tensor(
            "out", [B_CORE, P_TOTAL, EMB_DIM], OUT_DT, kind="ExternalOutput"
        ).ap()
        with tile.TileContext(nc) as tc:
            _bilinear_kernel(tc, out_ap, wt_aps, embt_aps, embn_aps)
        nc.compile()
        _CACHE["nc"] = nc
    return _CACHE["nc"]


def _pack_wt(W: np.ndarray):
    """W [496, 64, 64] fp32 -> per-group [128, gcols] fp16 tiles with the
    group's even fields on rows 0:64, odd on 64:128 (field order, padded);
    within a field, wt[64*(f%2)+d, base + lcol + (j-f-1)*64 + e] =
    W[p(f,j), e, d]. Group 0's leading WT_BASE[0] cols are left zero for
    the per-core chunk-0 embt head block."""
    Wh = W.astype(np.float16)
    outs = []
    for k, g in enumerate(WT_GROUPS):
        arr = np.zeros((128, WT_GCOLS[k]), dtype=np.float16)
        for f in g:
            r0 = 64 * (f % 2)
            c0 = WT_BASE[k] + WT_LCOL[f]
            blk = (
                Wh[OFF[f] : OFF[f + 1]]
                .transpose(2, 0, 1)
                .reshape(EMB_DIM, _cols(f))
            )
            arr[r0 : r0 + 64, c0 : c0 + _cols(f)] = blk
        outs.append(arr)
    return outs


def _pack_core_inputs(emb_shard: np.ndarray):
    """emb_shard [256, 32, 64] fp32 -> (embt0 [128, 2048], embt1, embn0,
    embn1). embt: field f at rows 64*(f%2):+64, cols (f//2)*128:+128, value
    emb[., f, .].T; embn: [128, 2048] row-major."""
    embts, embns = [], []
    for c in range(N_BCHUNK):
        chunk = emb_shard[c * 128 : (c + 1) * 128].astype(np.float16)
        et = np.zeros((128, NUM_FIELDS // 2 * 128), dtype=np.float16)
        for f in range(NUM_FIELDS):
            r0 = 64 * (f % 2)
            b0 = (f // 2) * 128
            et[r0 : r0 + 64, b0 : b0 + 128] = chunk[:, f, :].T
        embts.append(et)
        embns.append(
            np.ascontiguousarray(chunk.reshape(128, NUM_FIELDS * EMB_DIM))
        )
    return embts[0], embts[1], embns[0], embns[1]


def build_in_maps(feature_emb: np.ndarray, W: np.ndarray):
    wt_chunks = _pack_wt(np.asarray(W))
    emb = np.asarray(feature_emb, dtype=np.float32)
    hb = EMBT_HEAD_BLOCKS * 128
    in_maps = []
    for i in range(N_CORES):
        e0, e1, n0, n1 = _pack_core_inputs(emb[i * B_CORE : (i + 1) * B_CORE])
        prefix = wt_chunks[0].copy()
        prefix[:, :hb] = e0[:, :hb]
        m = {"embt0t": np.ascontiguousarray(e0[:, hb:]), "embt1": e1,
             "embn0": n0, "embn1": n1, "wt0": prefix}
        for k, w in enumerate(wt_chunks[1:], start=1):
            m[f"wt{k}"] = w
        in_maps.append(m)
    return in_maps


def run(feature_emb: np.ndarray, W: np.ndarray, trace: bool = False, tmpdir=None):
    """Returns (out [2048, 496, 64] fp32, BassKernelResults)."""
    nc = _get_program()
    in_maps = build_in_maps(feature_emb, W)
    res = bass_utils.run_bass_kernel_spmd(
        nc, in_maps, core_ids=list(range(N_CORES)), trace=trace, tmpdir=tmpdir
    )
    out = np.concatenate(
        [res.results[i]["out"] for i in range(N_CORES)], axis=0
    ).astype(np.float32)
    return out, res


def kernel(feature_emb: np.ndarray, W: np.ndarray) -> np.ndarray:
    out, _ = run(feature_emb, W)
    return out
